# revision 65
# baseline (speedup 1.0000x reference)
"""TRN2 Bass kernel for nn_Block1_43542378447225.

Pipeline (per sample, one NeuronCore; batch=2 -> cores 0/1 do real work):
  conv1 -> relu -> conv2 (direct from padded a1p views) -> relu
  -> Hopfield(z2) (transposed attention, ones-column softmax denom)
  -> backward g1 split: w2b @ z2 accumulates early, w2b @ (-q*m2) completes it
  -> mask -> C -> blocked e_min (9 shift-placed matmuls into one bf16 PSUM
  slot bank + one strided min-reduce) -> window mask -> masked patch forward
  -> Hopfield(z2_masked) -> output (written [pq, e]; host transposes)

Everything bf16 on device except PSUM accumulation (fp32) and the final
normalize (fp32 out). Host precomputes im2col patches, all weight layouts,
and K @ Vw, packed into deadline-ordered bf16 DMA blocks (7 input DMAs,
earliest-needed first; HWDGE triggers + the shared DMA bus serialize).

Layout conventions:
  pq = p*8+q (64 output positions), uv = u*10+v (100 composite-window offsets)
  hid chunk t = conv2 kernel row kr, a = conv2 kernel col ks
  kc = t*128 + a*32 + c1 (hidden index, 4 chunks of 128 partitions)
"""
import numpy as np
import ml_dtypes

import concourse.bass as bass
import concourse.bacc as bacc
import concourse.mybir as mybir
import concourse.tile as tile
from concourse.bass_utils import run_bass_kernel_spmd

F32 = mybir.dt.float32
BF16 = mybir.dt.bfloat16
AF = mybir.ActivationFunctionType
ALU = mybir.AluOpType

N_CORES = 8
BETA = 0.125  # 1/sqrt(64)
BF = ml_dtypes.bfloat16

_CACHE = {}

# wKT column map ([128, 772]): KT (rows 0:64) | KV_ext [128, 4, 65]
_KT0, _KV0, _KT_N = 0, 512, 772
# wEb2 column map: w2b | ident
_W2B0, _ID0, _EB2_N = 0, 512, 576
# wL1 column map: Scomb | PermF
_SC0, _PF0, _WL1_N = 0, 400, 544
# wL2 column map: PermB | W1big | w2fA
_PB0, _W1B0, _W2F0, _WL2_N = 0, 900, 2436, 2692


# ---------------------------------------------------------------- host prep
def _build_scomb_w1big(w1):
    w1s = w1.sum(axis=1)
    Scomb = np.zeros((4, 32, 4, 100), np.float32)  # [a, c1, t, uv]
    W1big = np.zeros((100, 3, 4, 4, 32), np.float32)  # [uv, h, t, a, c1]
    for t in range(4):
        for a in range(4):
            for u in range(10):
                ki = u - 2 * t
                if not (0 <= ki < 4):
                    continue
                for v in range(10):
                    kj = v - 2 * a
                    if not (0 <= kj < 4):
                        continue
                    Scomb[a, :, t, u * 10 + v] = w1s[:, ki, kj]
                    W1big[u * 10 + v, :, t, a, :] = w1[:, :, ki, kj].T
    return Scomb.reshape(128, 400), W1big.reshape(100, 1536)


def _host_prep(w1, b1, w2, b2, K, Vw):
    # wEa [32, 1024]: conv2 weights as 16 lhsT tiles [c1, (kr,ks), o]
    wEa = np.ascontiguousarray(
        np.transpose(w2, (1, 2, 3, 0)).reshape(32, 1024)).astype(BF)

    wKT = np.zeros((128, _KT_N), np.float32)
    wKT[0:64, _KT0:_KT0 + 512] = K.T
    KV = (K @ Vw).reshape(4, 128, 64)  # [t, r, e]
    kve = np.ones((128, 4, 65), np.float32)
    for t in range(4):
        kve[:, t, 0:64] = KV[t]
    wKT[:, _KV0:_KV0 + 260] = kve.reshape(128, 260)

    wEb2 = np.zeros((64, _EB2_N), np.float32)
    wEb2[:, _W2B0:_W2B0 + 512] = 2.0 * np.transpose(
        w2, (0, 2, 3, 1)).reshape(64, 512)
    wEb2[:, _ID0:_ID0 + 64] = np.eye(64, dtype=np.float32)

    Scomb, W1big = _build_scomb_w1big(w1)
    PermF = np.zeros((100, 9, 16), np.float32)
    for k in range(9):
        dp, dq = k // 3 - 1, k % 3 - 1
        for im in range(4):
            u = 4 * dp + im + 3
            if not (0 <= u < 10):
                continue
            for jm in range(4):
                v = 4 * dq + jm + 3
                if not (0 <= v < 10):
                    continue
                PermF[u * 10 + v, k, im * 4 + jm] = 1.0
    PermB = np.transpose(PermF, (2, 1, 0)).reshape(16, 900)

    wL1 = np.zeros((128, _WL1_N), np.float32)
    wL1[:, _SC0:_SC0 + 400] = Scomb
    wL1[0:100, _PF0:_PF0 + 144] = PermF.reshape(100, 144)

    wL2 = np.zeros((128, _WL2_N), np.float32)
    wL2[0:16, _PB0:_PB0 + 900] = PermB
    wL2[0:100, _W1B0:_W1B0 + 1536] = W1big
    wL2[:, _W2F0:_W2F0 + 256] = np.transpose(w2, (3, 1, 2, 0)).reshape(128, 256)

    return {"wEa": wEa, "wKT": wKT.astype(BF), "wEb2": wEb2.astype(BF),
            "wL1": wL1.astype(BF), "wL2": wL2.astype(BF),
            "_w1f": np.ascontiguousarray(
                np.transpose(w1, (2, 3, 1, 0)).reshape(48, 32)),
            "_b1": b1, "_b2": b2}


def _sample_prep(x_s, w1f, b1, b2):
    xp1 = np.pad(x_s, ((0, 0), (1, 1), (1, 1)))
    xp3 = np.pad(x_s, ((0, 0), (3, 3), (3, 3)))
    P1 = np.zeros((4, 4, 3, 16, 16), np.float32)
    for kr in range(4):
        for ks in range(4):
            P1[kr, ks] = xp1[:, kr:kr + 32:2, ks:ks + 32:2][:, :16, :16]
    X = np.zeros((10, 10, 3, 8, 8), np.float32)
    for u in range(10):
        for v in range(10):
            X[u, v] = xp3[:, u:u + 32:4, v:v + 32:4][:, :8, :8]
    cvb = np.zeros((64, 290), np.float32)
    cvb[0:48, 0:256] = P1.reshape(48, 256)
    cvb[0:48, 256:288] = w1f
    cvb[0:32, 288] = b1
    cvb[0:64, 289] = b2
    return cvb.astype(BF), X.reshape(100, 192).astype(BF)


# ---------------------------------------------------------------- device build
def _hf(nc, sb, ps, z_sb, wKT, tag):
    """Transposed Hopfield: z [64c, 64pq] -> q_ps [64pq, 65] (col 64 = softmax
    denominator via the ones column baked into KV_ext), rec [64pq, 1] = 1/denom.
    Scores bounded (|beta*S| small) -> softmax skips max subtraction.
    q matmuls run as 8 half-chunk contractions so KV can ride in the
    64-partition wKT block (att partition halves as lhsT)."""
    S_ps = ps.tile([128, 4, 64], F32, tag="psC", bufs=1, name=f"S{tag}")
    for t in range(4):
        nc.tensor.matmul(S_ps[:, t, :],
                         wKT[0:64, _KT0 + t * 128:_KT0 + (t + 1) * 128],
                         z_sb, start=True, stop=True)
    att = sb.tile([128, 4, 64], BF16, tag=f"att{tag}", name=f"att{tag}")
    nc.scalar.activation(out=att[:], in_=S_ps[:], func=AF.Exp,
                         bias=0.0, scale=BETA)
    q_ps = ps.tile([64, 65], F32, tag="psD", bufs=1, name=f"q{tag}")
    for t in range(4):
        off = _KV0 + t * 65
        nc.tensor.matmul(q_ps[:], att[:, t, :], wKT[:, off:off + 65],
                         start=(t == 0), stop=(t == 3))
    rec = sb.tile([64, 1], F32, tag=f"rec{tag}", name=f"rec{tag}")
    nc.vector.reciprocal(rec[:], q_ps[:, 64:65])
    return q_ps, rec


def _build_nc(debug=False):
    nc = bacc.Bacc("TRN2", target_bir_lowering=False, debug=False,
                   num_devices=N_CORES)
    d_cvb = nc.dram_tensor("cvb", [64, 290], BF16, kind="ExternalInput")
    d_wEa = nc.dram_tensor("wEa", [32, 1024], BF16, kind="ExternalInput")
    d_wKT = nc.dram_tensor("wKT", [128, _KT_N], BF16, kind="ExternalInput")
    d_wEb2 = nc.dram_tensor("wEb2", [64, _EB2_N], BF16, kind="ExternalInput")
    d_smpl = nc.dram_tensor("smpl", [100, 192], BF16, kind="ExternalInput")
    d_wL1 = nc.dram_tensor("wL1", [128, _WL1_N], BF16, kind="ExternalInput")
    d_wL2 = nc.dram_tensor("wL2", [128, _WL2_N], BF16, kind="ExternalInput")
    out_t = nc.dram_tensor("out", [64, 64], F32, kind="ExternalOutput")
    probes = {}

    def probe(name, shape):
        if debug:
            probes[name] = nc.dram_tensor("probe_" + name, shape, F32,
                                          kind="ExternalOutput")
        return probes.get(name)

    with tile.TileContext(nc) as tc:
        with tc.tile_pool(name="sb", bufs=1) as sb, \
             tc.tile_pool(name="ps", bufs=1, space="PSUM") as ps:
            # ---- PE warm-up ASAP (Pool memset is ready earliest) so the
            # PE p-state ramp (3us to max clock) starts before conv1.
            warm = sb.tile([2, 8], F32, tag="warm")
            nc.gpsimd.memset(warm[:], 0.0)
            for w_ in range(2):
                warm_ps = ps.tile([8, 8], F32, tag="psQ1C", bufs=1,
                                  name=f"warm{w_}")
                nc.tensor.matmul(warm_ps[:], warm[:], warm[:],
                                 start=True, stop=True)

            # ---- loads, earliest deadline first.
            # SP queue: cvb, wKT, smpl, wL2; ACT queue: wEa, wEb2, wL1
            cvb = sb.tile([64, 290], BF16, tag="cvb")
            nc.sync.dma_start(out=cvb[:], in_=d_cvb[:])
            wEa = sb.tile([32, 1024], BF16, tag="wEa")
            nc.scalar.dma_start(out=wEa[:], in_=d_wEa[:])
            wKT = sb.tile([128, _KT_N], BF16, tag="wKT")
            nc.sync.dma_start(out=wKT[:], in_=d_wKT[:])
            wEb2 = sb.tile([64, _EB2_N], BF16, tag="wEb2")
            nc.sync.dma_start(out=wEb2[:], in_=d_wEb2[:])
            wL1 = sb.tile([128, _WL1_N], BF16, tag="wL1")
            nc.sync.dma_start(out=wL1[:], in_=d_wL1[:])
            wL2 = sb.tile([128, _WL2_N], BF16, tag="wL2")
            nc.sync.dma_start(out=wL2[:], in_=d_wL2[:])
            smpl = sb.tile([100, 3, 64], BF16, tag="smpl")
            nc.sync.dma_start(out=smpl[:], in_=d_smpl[:].rearrange(
                "u (h q) -> u h q", h=3))

            # ---- early zero fills (a1p border, e_min acc margins, candidate
            # PSUM slot bank whose unwritten regions act as the 0-candidate)
            a1p = sb.tile([32, 18, 18], BF16, tag="a1p")
            nc.gpsimd.memset(a1p[:], 0.0)
            acc = sb.tile([16, 12, 8], BF16, tag="acc")
            nc.gpsimd.memset(acc[:], 0.0)
            cnd = ps.tile([16, 9, 64], F32, tag="psCND", bufs=1,
                          name="cnd")
            nc.vector.memset(cnd[:], 0.0)

            # biases as fp32 (tensor_scalar requires fp32 scalar operands)
            bias_f = sb.tile([64, 2], F32, tag="bias_f")
            nc.vector.tensor_copy(out=bias_f[:], in_=cvb[0:64, 288:290])

            # ---- conv1 + relu into padded a1p [32, 18, 18]
            a1_ps = ps.tile([32, 256], F32, tag="psA", bufs=1)
            nc.tensor.matmul(a1_ps[:], cvb[0:48, 256:288], cvb[0:48, 0:256],
                             start=True, stop=True)
            nc.vector.tensor_scalar(
                out=a1p[:, 1:17, 1:17],
                in0=a1_ps[:].rearrange("c (p q) -> c p q", p=16),
                scalar1=bias_f[0:32, 0:1], scalar2=0.0,
                op0=ALU.add, op1=ALU.max)

            # ---- conv2 + relu -> z2 [64, 64]: 16 matmuls straight off a1p
            a1p_ap = a1p[:]
            z2_ps = ps.tile([64, 64], F32, tag="psB", bufs=1)
            for kr in range(4):
                for ks in range(4):
                    rhs = bass.AP(tensor=a1p_ap.tensor,
                                  offset=a1p_ap.offset + kr * 18 + ks,
                                  ap=[[324, 32], [36, 8], [2, 8]])
                    nc.tensor.matmul(
                        z2_ps[:],
                        wEa[:, (kr * 4 + ks) * 64:(kr * 4 + ks + 1) * 64],
                        rhs, start=(kr == 0 and ks == 0),
                        stop=(kr == 3 and ks == 3))
            z2 = sb.tile([64, 64], BF16, tag="z2")
            nc.vector.tensor_scalar(out=z2[:], in0=z2_ps[:],
                                    scalar1=bias_f[:, 1:2], scalar2=0.0,
                                    op0=ALU.add, op1=ALU.max)
            if debug:
                z2f = sb.tile([64, 64], F32, tag="z2f")
                nc.vector.tensor_copy(out=z2f[:], in_=z2[:])
                nc.sync.dma_start(out=probe("z2", [64, 64])[:], in_=z2f[:])

            # ---- relu masks (all DVE, in the hopfield shadow):
            # m2 {0,1} for z2m; m2n {0,-1} folds the subtraction into g1;
            # M1W (P2-layout conv1 mask) from strided a1p views
            m2 = sb.tile([64, 64], BF16, tag="m2")
            nc.vector.tensor_scalar(out=m2[:], in0=z2[:], scalar1=0.0,
                                    scalar2=None, op0=ALU.not_equal)
            m2n = sb.tile([64, 64], BF16, tag="m2n")
            nc.vector.tensor_scalar(out=m2n[:], in0=z2[:], scalar1=0.0,
                                    scalar2=-1.0, op0=ALU.not_equal,
                                    op1=ALU.mult)
            # ---- Hopfield #1 -> qm = (q/denom)^T * (-m2)
            q1_ps, rec1 = _hf(nc, sb, ps, z2[:], wKT, "1")

            # conv1 mask in the hopfield shadow (issued late so relu2/m2
            # win the DVE when conv2 lands)
            M1W = sb.tile([128, 4, 64], BF16, tag="M1W")
            for a in range(4):
                src = bass.AP(tensor=a1p_ap.tensor, offset=a1p_ap.offset + a,
                              ap=[[324, 32], [18, 4], [36, 8], [2, 8]])
                nc.vector.tensor_scalar(out=M1W[a * 32:(a + 1) * 32, :, :],
                                        in0=src, scalar1=0.0,
                                        scalar2=None, op0=ALU.not_equal)

            # ---- g1 first half: w2b^T @ z2 accumulates in the hopfield
            # shadow (issued after the S matmuls so exp1's input wins the PE)
            g1_ps = ps.tile([128, 4, 64], F32, tag="psG", bufs=1, name="g1")
            for t in range(4):
                nc.tensor.matmul(g1_ps[:, t, :],
                                 wEb2[:, _W2B0 + t * 128:_W2B0 + (t + 1) * 128],
                                 z2[:], start=True, stop=False)
            qn1T = sb.tile([64, 64], BF16, tag="qn1T")
            nc.vector.tensor_scalar_mul(qn1T[:], q1_ps[:, 0:64], rec1[:])
            q1c_ps = ps.tile([64, 64], BF16, tag="psQ1C", bufs=1, name="q1c")
            nc.tensor.transpose(q1c_ps[:], qn1T[:], wEb2[:, _ID0:_ID0 + 64])
            qm = sb.tile([64, 64], BF16, tag="qm")
            nc.vector.tensor_tensor(out=qm[:], in0=q1c_ps[:], in1=m2n[:],
                                    op=ALU.mult)

            # ---- g1 second half (the masked -q term) -> g1m
            for t in range(4):
                nc.tensor.matmul(g1_ps[:, t, :],
                                 wEb2[:, _W2B0 + t * 128:_W2B0 + (t + 1) * 128],
                                 qm[:], start=False, stop=True)
            g1m = sb.tile([128, 4, 64], BF16, tag="g1m")
            nc.vector.tensor_tensor(out=g1m[:], in0=g1_ps[:], in1=M1W[:],
                                    op=ALU.mult)

            # ---- C [100, 64] = sum_t Scomb_t^T @ g1m_t
            C_ps = ps.tile([100, 64], F32, tag="psB", bufs=1, name="C")
            for t in range(4):
                nc.tensor.matmul(C_ps[:],
                                 wL1[:, _SC0 + t * 100:_SC0 + (t + 1) * 100],
                                 g1m[:, t, :], start=(t == 0), stop=(t == 3))
            C_sb = sb.tile([100, 8, 8], BF16, tag="C_sb")
            nc.vector.tensor_copy(
                out=C_sb[:], in_=C_ps[:].rearrange("u (p q) -> u p q", p=8))
            if debug:
                Cf = sb.tile([100, 64], F32, tag="Cf")
                nc.vector.tensor_copy(out=Cf[:], in_=C_ps[:])
                nc.sync.dma_start(out=probe("C", [100, 64])[:], in_=Cf[:])

            # ---- blocked e_min: 9 shift-placed permutation matmuls into the
            # pre-zeroed bf16 slot bank, then one strided min-reduce
            cnd_v = cnd[:].rearrange("k s (p q) -> k s p q", p=8)
            for k in range(9):
                dp, dq = k // 3 - 1, k % 3 - 1
                ilo, ihi = max(0, dp), min(8, 8 + dp)
                jlo, jhi = max(0, dq), min(8, 8 + dq)
                nc.tensor.matmul(
                    cnd_v[:, k, ilo:ihi, jlo:jhi],
                    wL1[0:100, _PF0 + k * 16:_PF0 + (k + 1) * 16],
                    C_sb[:, ilo - dp:ihi - dp, jlo - dq:jhi - dq],
                    start=(k == 0), stop=(k == 8), skip_group_check=True)
            nc.vector.tensor_reduce(
                out=acc[:, 2:10, :], in_=cnd[:].rearrange("k s q -> k q s"),
                axis=mybir.AxisListType.X, op=ALU.min)

            # ---- eW (e_min in window form) + mask + masked patches
            accf = acc[:].rearrange("a b c -> a (b c)")
            eW_ps = ps.tile([100, 64], F32, tag="psD", bufs=1, name="eW")
            for k in range(9):
                dp, dq = k // 3 - 1, k % 3 - 1
                off = 16 + 8 * dp + dq
                nc.tensor.matmul(eW_ps[:],
                                 wL2[0:16, _PB0 + k * 100:_PB0 + (k + 1) * 100],
                                 accf[:, off:off + 64],
                                 start=(k == 0), stop=(k == 8))
            maskw = sb.tile([100, 64], BF16, tag="maskw")
            nc.vector.tensor_tensor(
                out=maskw[:], in0=C_sb[:].rearrange("u p q -> u (p q)"),
                in1=eW_ps[:], op=ALU.is_le)
            if debug:
                mf = sb.tile([100, 64], F32, tag="mf")
                nc.vector.tensor_copy(out=mf[:], in_=maskw[:])
                nc.sync.dma_start(out=probe("maskw", [100, 64])[:], in_=mf[:])
            Xm_h = [sb.tile([100, 64], BF16, tag=f"Xm{h}", name=f"Xm{h}")
                    for h in range(3)]
            for h in range(3):
                nc.vector.tensor_tensor(out=Xm_h[h][:], in0=smpl[:, h, :],
                                        in1=maskw[:], op=ALU.mult)

            # ---- masked forward: u1m, zm, z2m
            u1_ps = ps.tile([128, 4, 64], F32, tag="psA", bufs=1, name="u1")
            for h in range(3):
                for t in range(4):
                    nc.tensor.matmul(
                        u1_ps[:, t, :],
                        wL2[0:100, _W1B0 + h * 512 + t * 128:
                            _W1B0 + h * 512 + (t + 1) * 128],
                        Xm_h[h][:], start=(h == 0), stop=(h == 2))
            u1m = sb.tile([128, 4, 64], BF16, tag="u1m")
            nc.vector.tensor_tensor(out=u1m[:], in0=u1_ps[:], in1=M1W[:],
                                    op=ALU.mult)
            zm_ps = ps.tile([64, 64], F32, tag="psB", bufs=1, name="zm")
            for t in range(4):
                nc.tensor.matmul(zm_ps[:],
                                 wL2[:, _W2F0 + t * 64:_W2F0 + (t + 1) * 64],
                                 u1m[:, t, :], start=(t == 0), stop=(t == 3))
            z2m = sb.tile([64, 64], BF16, tag="z2m")
            nc.vector.tensor_tensor(out=z2m[:], in0=zm_ps[:], in1=m2[:],
                                    op=ALU.mult)
            if debug:
                zmf = sb.tile([64, 64], F32, tag="zmf")
                nc.vector.tensor_copy(out=zmf[:], in_=z2m[:])
                nc.sync.dma_start(out=probe("z2m", [64, 64])[:], in_=zmf[:])

            # ---- Hopfield #2 -> output [pq, e] (host transposes)
            q2_ps, rec2 = _hf(nc, sb, ps, z2m[:], wKT, "2")
            out_sb = sb.tile([64, 64], F32, tag="out_sb")
            nc.vector.tensor_scalar_mul(out_sb[:], q2_ps[:, 0:64], rec2[:])
            nc.sync.dma_start(out=out_t[:], in_=out_sb[:])
    nc.compile()
    return nc


def _get_nc(debug=False):
    key = ("nc", debug)
    if key not in _CACHE:
        _CACHE[key] = _build_nc(debug)
    return _CACHE[key]


# ---------------------------------------------------------------- entry point
def kernel(x, w1, b1, w2, b2, K, Vw, _debug=False):
    x = np.asarray(x, np.float32)
    shared = _host_prep(np.asarray(w1, np.float32), np.asarray(b1, np.float32),
                        np.asarray(w2, np.float32), np.asarray(b2, np.float32),
                        np.asarray(K, np.float32), np.asarray(Vw, np.float32))
    w1f = shared.pop("_w1f")
    b1h, b2h = shared.pop("_b1"), shared.pop("_b2")
    bsz = x.shape[0]
    nc = _get_nc(_debug)
    smpls = [_sample_prep(x[b], w1f, b1h, b2h) for b in range(bsz)]
    in_maps = []
    for core in range(N_CORES):
        cvb, xb = smpls[core] if core < bsz else smpls[0]
        m = dict(shared)
        m["cvb"], m["smpl"] = cvb, xb
        in_maps.append(m)
    res = run_bass_kernel_spmd(nc, in_maps, core_ids=list(range(N_CORES)))
    out = np.stack([res.results[b]["out"].astype(np.float32).T.reshape(64, 8, 8)
                    for b in range(bsz)])
    if _debug:
        return out, res
    return out
